# revision 5
# baseline (speedup 1.0000x reference)
"""Cayley-circulant SSM layer as a Trainium2 Bass kernel.

Math: h_t = W h_{t-1} + B u_t, y_t = C h_t + D u_t, where W is a real
orthogonal circulant (Cayley transform of a skew-circulant) diagonalized
by the DFT with unit-modulus eigenvalues lambda_k = e^{i theta_k}.

Device algorithm (frequency-domain associative scan):
  1. Fold the rfft into B and C on the host (weight preprocessing):
     buhat_t = (F B) u_t restricted to 512 packed real frequency
     channels (Hermitian symmetry; DC and Nyquist share channel 0 as
     (re, im) with theta=0).
  2. The recurrence hhat_t = lambda * hhat_{t-1} + buhat_t becomes,
     with z_t = conj(lambda)^t * buhat_t, a plain cumulative sum:
     hhat_t = lambda^t * cumsum(z)_t.  |lambda|=1 so this is exact.
  3. y_t = Re(G hhat_t) = Ar @ hhat_r + Ai @ hhat_i + D u_t.

Per-core layout (data-parallel over batch, 1 row per NeuronCore):
  MM1  (PE, fp32r):  bu_{r,i}(k,t) = BrT/BiT.T @ uT          (d contracted)
  twist (DVE):       m1 = c*bur, m2 = s*bui, m3 = c*bui, m4 = s*bur
  scan  (DVE):       Sr = cumsum(m1+m2), Si = cumsum(m3-m4)   (fused)
  untwist (DVE):     hr = c*Sr - s*Si, hi = c*Si + s*Sr       (fp32r out)
  MM3  (PE, fp32r):  yT(d,t) = ArT.T @ hr + AiT.T @ hi        (k contracted)
cos/sin tables are host-precomputed in float64 per (k, t).
"""

import numpy as np

import concourse.bass as bass  # noqa: F401  (registers engine types)
import concourse.mybir as mybir
import concourse.tile as tile
from concourse import bacc
from concourse.bass_utils import run_bass_kernel_spmd

BATCH, SEQ, DM, NSTATE = 8, 2048, 1024, 1024
K = NSTATE // 2          # packed real frequency channels
P = 128                  # partitions
TT = 512                 # t-tile width (one PSUM bank of fp32)
ND, NK, NT = DM // P, K // P, SEQ // TT

_f32 = mybir.dt.float32
_f32r = mybir.dt.float32r
_add = mybir.AluOpType.add
_sub = mybir.AluOpType.subtract
_mul = mybir.AluOpType.mult

_COMPILED = None


def _emit(tc, nc, dr):
    ut_d, brt_d, bit_d, art_d, ait_d, cs_d, sn_d, yt_d = dr
    with (
        tc.tile_pool(name="ust", bufs=2) as ust,
        tc.tile_pool(name="wb", bufs=1) as wb,
        tc.tile_pool(name="wa", bufs=2) as wa,
        tc.tile_pool(name="tbl", bufs=3) as tbl,
        tc.tile_pool(name="hbuf", bufs=1) as hbuf,
        tc.tile_pool(name="scr", bufs=2) as scr,
        tc.tile_pool(name="sbuf_s", bufs=2) as sbuf_s,
        tc.tile_pool(name="ini", bufs=2) as ini,
        tc.tile_pool(name="psA", bufs=2, space="PSUM") as psA,
        tc.tile_pool(name="psY", bufs=4, space="PSUM") as psY,
    ):
        # resident: B-projection weights (4 MB)
        brt, bit = [], []
        for di in range(ND):
            tb = wb.tile([P, K], _f32r, tag=f"brt{di}")
            nc.sync.dma_start(tb[:], brt_d[di])
            brt.append(tb)
            ti = wb.tile([P, K], _f32r, tag=f"bit{di}")
            nc.sync.dma_start(ti[:], bit_d[di])
            bit.append(ti)

        # persistent untwisted spectra for MM3: 8 MB
        hr = [hbuf.tile([P, SEQ], _f32r, name=f"hr{kt}", tag=f"hr{kt}")
              for kt in range(NK)]
        hi = [hbuf.tile([P, SEQ], _f32r, name=f"hi{kt}", tag=f"hi{kt}")
              for kt in range(NK)]
        # per-k-tile scan carry (last cumsum column of the previous t-tile)
        ir = [ini.tile([P, 1], _f32, name=f"ir{kt}", tag=f"ir{kt}")
              for kt in range(NK)]
        ii = [ini.tile([P, 1], _f32, name=f"ii{kt}", tag=f"ii{kt}")
              for kt in range(NK)]

        # ---- phase A: MM1 + twist + scan + untwist ----
        for tt in range(NT):
            ts = slice(tt * TT, (tt + 1) * TT)
            us = []
            for di in range(ND):
                t = ust.tile([P, TT], _f32r, tag=f"us{di}")
                nc.sync.dma_start(t[:], ut_d[di, :, ts])
                us.append(t)
            for kt in range(NK):
                pbr = psA.tile([P, TT], _f32, tag="pbr")
                pbi = psA.tile([P, TT], _f32, tag="pbi")
                for di in range(ND):
                    lhs_r = brt[di][:, kt * P:(kt + 1) * P]
                    lhs_i = bit[di][:, kt * P:(kt + 1) * P]
                    nc.tensor.matmul(pbr[:], lhs_r, us[di][:],
                                     start=(di == 0), stop=(di == ND - 1))
                    nc.tensor.matmul(pbi[:], lhs_i, us[di][:],
                                     start=(di == 0), stop=(di == ND - 1))
                c = tbl.tile([P, TT], _f32, tag="cs")
                s = tbl.tile([P, TT], _f32, tag="sn")
                nc.sync.dma_start(c[:], cs_d[kt, :, ts])
                nc.sync.dma_start(s[:], sn_d[kt, :, ts])

                m1 = scr.tile([P, TT], _f32, tag="m1")
                m2 = scr.tile([P, TT], _f32, tag="m2")
                nc.vector.tensor_tensor(m1[:], c[:], pbr[:], _mul)
                nc.vector.tensor_tensor(m2[:], s[:], pbi[:], _mul)
                sr = sbuf_s.tile([P, TT], _f32, tag="sr")
                init_r = 0.0 if tt == 0 else ir[kt][:]
                nc.vector.tensor_tensor_scan(sr[:], m1[:], m2[:], init_r,
                                             _add, _add)
                m3 = scr.tile([P, TT], _f32, tag="m1")
                m4 = scr.tile([P, TT], _f32, tag="m2")
                nc.vector.tensor_tensor(m3[:], c[:], pbi[:], _mul)
                nc.vector.tensor_tensor(m4[:], s[:], pbr[:], _mul)
                si = sbuf_s.tile([P, TT], _f32, tag="si")
                init_i = 0.0 if tt == 0 else ii[kt][:]
                nc.vector.tensor_tensor_scan(si[:], m3[:], m4[:], init_i,
                                             _add, _sub)
                if tt < NT - 1:
                    nc.scalar.copy(ir[kt][:], sr[:, TT - 1:TT])
                    nc.scalar.copy(ii[kt][:], si[:, TT - 1:TT])

                w1 = scr.tile([P, TT], _f32, tag="w1")
                w2 = scr.tile([P, TT], _f32, tag="w2")
                nc.vector.tensor_tensor(w1[:], c[:], sr[:], _mul)
                nc.vector.tensor_tensor(w2[:], s[:], si[:], _mul)
                nc.vector.tensor_tensor(hr[kt][:, ts], w1[:], w2[:], _sub)
                w3 = scr.tile([P, TT], _f32, tag="w1")
                w4 = scr.tile([P, TT], _f32, tag="w2")
                nc.vector.tensor_tensor(w3[:], c[:], si[:], _mul)
                nc.vector.tensor_tensor(w4[:], s[:], sr[:], _mul)
                nc.vector.tensor_tensor(hi[kt][:, ts], w3[:], w4[:], _add)

        # ---- phase B: MM3 (di-outer so A-weight slices stream once) ----
        for di in range(ND):
            ar_s, ai_s = [], []
            for kt in range(NK):
                ta = wa.tile([P, P], _f32r, tag=f"ars{kt}")
                nc.sync.dma_start(ta[:], art_d[kt, :, di * P:(di + 1) * P])
                ar_s.append(ta)
                ti = wa.tile([P, P], _f32r, tag=f"ais{kt}")
                nc.sync.dma_start(ti[:], ait_d[kt, :, di * P:(di + 1) * P])
                ai_s.append(ti)
            for tt in range(NT):
                ts = slice(tt * TT, (tt + 1) * TT)
                py = psY.tile([P, TT], _f32, tag="py")
                for kt in range(NK):
                    nc.tensor.matmul(py[:], ar_s[kt][:], hr[kt][:, ts],
                                     start=(kt == 0), stop=False)
                    nc.tensor.matmul(py[:], ai_s[kt][:], hi[kt][:, ts],
                                     start=False, stop=(kt == NK - 1))
                ysb = sbuf_s.tile([P, TT], _f32, tag="ysb", bufs=4)
                nc.scalar.copy(ysb[:], py[:])
                nc.sync.dma_start(yt_d[di, :, ts], ysb[:])


def _build():
    nc = bacc.Bacc("TRN2", target_bir_lowering=False, debug=False,
                   num_devices=BATCH)
    ut_d = nc.dram_tensor("ut", [ND, P, SEQ], _f32r, kind="ExternalInput")
    brt_d = nc.dram_tensor("brt", [ND, P, K], _f32r, kind="ExternalInput")
    bit_d = nc.dram_tensor("bit", [ND, P, K], _f32r, kind="ExternalInput")
    art_d = nc.dram_tensor("art", [NK, P, DM], _f32r, kind="ExternalInput")
    ait_d = nc.dram_tensor("ait", [NK, P, DM], _f32r, kind="ExternalInput")
    cs_d = nc.dram_tensor("cs", [NK, P, SEQ], _f32, kind="ExternalInput")
    sn_d = nc.dram_tensor("sn", [NK, P, SEQ], _f32, kind="ExternalInput")
    yt_d = nc.dram_tensor("yt", [ND, P, SEQ], _f32, kind="ExternalOutput")
    with tile.TileContext(nc) as tc:
        _emit(tc, nc, (ut_d, brt_d, bit_d, art_d, ait_d, cs_d, sn_d, yt_d))
    nc.compile()
    return nc


def _host_prep(a_params, B_w, C_w):
    """Fold the DFT into B/C and build phase tables (float64 on host)."""
    n, half = NSTATE, K
    a = a_params.astype(np.float64)
    a_full = np.zeros(n)
    a_full[1:half] = a[:half - 1]
    a_full[half + 1:] = -a[:half - 1][::-1]
    omega = np.fft.fft(a_full).imag
    theta = -2.0 * np.arctan(omega)          # lambda_k = e^{i theta_k}

    Bf = np.fft.fft(B_w.astype(np.float64), axis=0)[:half + 1]      # (513, d)
    G = np.conj(np.fft.fft(C_w.astype(np.float64), axis=1))[:, :half + 1]

    Br = np.empty((K, DM))
    Bi = np.empty((K, DM))
    Br[0], Bi[0] = Bf[0].real, Bf[half].real   # DC + Nyquist packed, theta=0
    Br[1:], Bi[1:] = Bf[1:half].real, Bf[1:half].imag
    Ar = np.empty((DM, K))
    Ai = np.empty((DM, K))
    Ar[:, 0] = (1.0 / n) * G[:, 0].real
    Ai[:, 0] = (1.0 / n) * G[:, half].real
    Ar[:, 1:] = (2.0 / n) * G[:, 1:half].real
    Ai[:, 1:] = -(2.0 / n) * G[:, 1:half].imag
    th = theta[:half].copy()
    th[0] = 0.0

    ang = np.outer(th, np.arange(SEQ, dtype=np.float64))   # (K, SEQ)
    f32 = np.float32

    def tiles(m, p):          # (R, C) -> (R//p, p, C) contiguous f32
        return np.ascontiguousarray(m.reshape(m.shape[0] // p, p, m.shape[1]),
                                    dtype=f32)

    return {
        "brt": tiles(Br.T.copy(), P),       # (8, 128, 512)  BrT[d, k]
        "bit": tiles(Bi.T.copy(), P),
        "art": tiles(Ar.T.copy(), P),       # (4, 128, 1024) ArT[k, d]
        "ait": tiles(Ai.T.copy(), P),
        "cs": tiles(np.cos(ang), P),        # (4, 128, 2048)
        "sn": tiles(np.sin(ang), P),
    }


def _run(u, a_params, B_w, C_w, D, trace=False):
    global _COMPILED
    if _COMPILED is None:
        _COMPILED = _build()
    nc = _COMPILED
    shared = _host_prep(np.asarray(a_params), np.asarray(B_w), np.asarray(C_w))
    u = np.asarray(u)
    in_maps = []
    for b in range(BATCH):
        m = dict(shared)
        m["ut"] = np.ascontiguousarray(
            u[b].T.reshape(ND, P, SEQ), dtype=np.float32)
        in_maps.append(m)
    res = run_bass_kernel_spmd(nc, in_maps, core_ids=list(range(BATCH)),
                               trace=trace)
    y = np.empty((BATCH, SEQ, DM), dtype=np.float32)
    for b in range(BATCH):
        y[b] = res.results[b]["yt"].reshape(DM, SEQ).T
    y += np.asarray(D)[None, None, :] * u
    return y, res


def kernel(u, a_params, B_w, C_w, D):
    y, _ = _run(u, a_params, B_w, C_w, D)
    return y


# revision 6
# speedup vs baseline: 1.1636x; 1.1636x over previous
"""Cayley-circulant SSM layer as a Trainium2 Bass kernel.

Math: h_t = W h_{t-1} + B u_t, y_t = C h_t + D u_t, where W is a real
orthogonal circulant (Cayley transform of a skew-circulant) diagonalized
by the DFT with unit-modulus eigenvalues lambda_k = e^{i theta_k}.

Device algorithm (frequency-domain associative scan):
  1. Fold the rfft into B and C on the host (weight preprocessing):
     buhat_t = (F B) u_t restricted to 512 packed real frequency
     channels (Hermitian symmetry; DC and Nyquist share channel 0 as
     (re, im) with theta=0).
  2. The recurrence hhat_t = lambda * hhat_{t-1} + buhat_t becomes,
     with z_t = conj(lambda)^t * buhat_t, a plain cumulative sum:
     hhat_t = lambda^t * cumsum(z)_t.  |lambda|=1 so this is exact.
  3. y_t = Re(G hhat_t) = Ar @ hhat_r + Ai @ hhat_i + D u_t.

Per-core layout (data-parallel over batch, 1 row per NeuronCore):
  MM1  (PE, fp32r):  bu_{r,i}(k,t) = BrT/BiT.T @ uT          (d contracted)
  twist (DVE):       m1 = c*bur, m2 = s*bui, m3 = c*bui, m4 = s*bur
  scan  (DVE):       Sr = cumsum(m1+m2), Si = cumsum(m3-m4)   (fused)
  untwist (DVE):     hr = c*Sr - s*Si, hi = c*Si + s*Sr       (fp32r out)
  MM3  (PE, fp32r):  yT(d,t) = ArT.T @ hr + AiT.T @ hi        (k contracted)
cos/sin tables are host-precomputed in float64 per (k, t).
"""

import numpy as np

import concourse.bass as bass  # noqa: F401  (registers engine types)
import concourse.mybir as mybir
import concourse.tile as tile
from concourse import bacc
from concourse.bass_utils import run_bass_kernel_spmd

BATCH, SEQ, DM, NSTATE = 8, 2048, 1024, 1024
K = NSTATE // 2          # packed real frequency channels
P = 128                  # partitions
TT = 512                 # t-tile width (one PSUM bank of fp32)
ND, NK, NT = DM // P, K // P, SEQ // TT

_f32 = mybir.dt.float32
_f32r = mybir.dt.float32r
_add = mybir.AluOpType.add
_sub = mybir.AluOpType.subtract
_mul = mybir.AluOpType.mult

_COMPILED = None


def _emit(tc, nc, dr):
    ut_d, brt_d, bit_d, art_d, ait_d, cs_d, sn_d, yt_d = dr
    with (
        tc.tile_pool(name="ust", bufs=2) as ust,
        tc.tile_pool(name="wb", bufs=1) as wb,
        tc.tile_pool(name="wa", bufs=1) as wa,
        tc.tile_pool(name="tbl", bufs=3) as tbl,
        tc.tile_pool(name="hbuf", bufs=2) as hbuf,
        tc.tile_pool(name="scr", bufs=2) as scr,
        tc.tile_pool(name="sbuf_s", bufs=2) as sbuf_s,
        tc.tile_pool(name="ini", bufs=2) as ini,
        tc.tile_pool(name="psA", bufs=2, space="PSUM") as psA,
        tc.tile_pool(name="psY", bufs=4, space="PSUM") as psY,
    ):
        # resident weights: B-projection (4 MB) + A-projection (4 MB)
        brt, bit = [], []
        for di in range(ND):
            tb = wb.tile([P, K], _f32r, tag=f"brt{di}")
            nc.sync.dma_start(tb[:], brt_d[di])
            brt.append(tb)
            ti = wb.tile([P, K], _f32r, tag=f"bit{di}")
            nc.sync.dma_start(ti[:], bit_d[di])
            bit.append(ti)
        art, ait = [], []
        for kt in range(NK):
            ta = wa.tile([P, DM], _f32r, tag=f"art{kt}")
            nc.sync.dma_start(ta[:], art_d[kt])
            art.append(ta)
            ti = wa.tile([P, DM], _f32r, tag=f"ait{kt}")
            nc.sync.dma_start(ti[:], ait_d[kt])
            ait.append(ti)

        # per-k-tile scan carry (last cumsum column of the previous t-tile)
        ir = [ini.tile([P, 1], _f32, name=f"ir{kt}", tag=f"ir{kt}")
              for kt in range(NK)]
        ii = [ini.tile([P, 1], _f32, name=f"ii{kt}", tag=f"ii{kt}")
              for kt in range(NK)]

        # fused pipeline over t-tiles: MM1+twist+scan+untwist, then MM3
        for tt in range(NT):
            ts = slice(tt * TT, (tt + 1) * TT)
            us = []
            for di in range(ND):
                t = ust.tile([P, TT], _f32r, tag=f"us{di}")
                nc.sync.dma_start(t[:], ut_d[di, :, ts])
                us.append(t)
            hrt, hit = [], []
            for kt in range(NK):
                pbr = psA.tile([P, TT], _f32, tag="pbr")
                pbi = psA.tile([P, TT], _f32, tag="pbi")
                for di in range(ND):
                    lhs_r = brt[di][:, kt * P:(kt + 1) * P]
                    lhs_i = bit[di][:, kt * P:(kt + 1) * P]
                    nc.tensor.matmul(pbr[:], lhs_r, us[di][:],
                                     start=(di == 0), stop=(di == ND - 1))
                    nc.tensor.matmul(pbi[:], lhs_i, us[di][:],
                                     start=(di == 0), stop=(di == ND - 1))
                c = tbl.tile([P, TT], _f32, tag="cs")
                s = tbl.tile([P, TT], _f32, tag="sn")
                nc.sync.dma_start(c[:], cs_d[kt, :, ts])
                nc.sync.dma_start(s[:], sn_d[kt, :, ts])

                m1 = scr.tile([P, TT], _f32, tag="m1")
                m2 = scr.tile([P, TT], _f32, tag="m2")
                nc.vector.tensor_tensor(m1[:], c[:], pbr[:], _mul)
                nc.vector.tensor_tensor(m2[:], s[:], pbi[:], _mul)
                sr = sbuf_s.tile([P, TT], _f32, tag="sr")
                init_r = 0.0 if tt == 0 else ir[kt][:]
                nc.vector.tensor_tensor_scan(sr[:], m1[:], m2[:], init_r,
                                             _add, _add)
                m3 = scr.tile([P, TT], _f32, tag="m1")
                m4 = scr.tile([P, TT], _f32, tag="m2")
                nc.vector.tensor_tensor(m3[:], c[:], pbi[:], _mul)
                nc.vector.tensor_tensor(m4[:], s[:], pbr[:], _mul)
                si = sbuf_s.tile([P, TT], _f32, tag="si")
                init_i = 0.0 if tt == 0 else ii[kt][:]
                nc.vector.tensor_tensor_scan(si[:], m3[:], m4[:], init_i,
                                             _add, _sub)
                if tt < NT - 1:
                    nc.scalar.copy(ir[kt][:], sr[:, TT - 1:TT])
                    nc.scalar.copy(ii[kt][:], si[:, TT - 1:TT])

                w1 = scr.tile([P, TT], _f32, tag="w1")
                w2 = scr.tile([P, TT], _f32, tag="w2")
                hrk = hbuf.tile([P, TT], _f32r, name=f"hr{kt}", tag=f"hr{kt}")
                hik = hbuf.tile([P, TT], _f32r, name=f"hi{kt}", tag=f"hi{kt}")
                nc.vector.tensor_tensor(w1[:], c[:], sr[:], _mul)
                nc.vector.tensor_tensor(w2[:], s[:], si[:], _mul)
                nc.vector.tensor_tensor(hrk[:], w1[:], w2[:], _sub)
                w3 = scr.tile([P, TT], _f32, tag="w1")
                w4 = scr.tile([P, TT], _f32, tag="w2")
                nc.vector.tensor_tensor(w3[:], c[:], si[:], _mul)
                nc.vector.tensor_tensor(w4[:], s[:], sr[:], _mul)
                nc.vector.tensor_tensor(hik[:], w3[:], w4[:], _add)
                hrt.append(hrk)
                hit.append(hik)

            for di in range(ND):
                py = psY.tile([P, TT], _f32, tag="py")
                for kt in range(NK):
                    nc.tensor.matmul(py[:], art[kt][:, di * P:(di + 1) * P],
                                     hrt[kt][:], start=(kt == 0), stop=False)
                    nc.tensor.matmul(py[:], ait[kt][:, di * P:(di + 1) * P],
                                     hit[kt][:], start=False,
                                     stop=(kt == NK - 1))
                ysb = sbuf_s.tile([P, TT], _f32, tag="ysb", bufs=4)
                nc.scalar.copy(ysb[:], py[:])
                nc.sync.dma_start(yt_d[di, :, ts], ysb[:])


def _build():
    nc = bacc.Bacc("TRN2", target_bir_lowering=False, debug=False,
                   num_devices=BATCH)
    ut_d = nc.dram_tensor("ut", [ND, P, SEQ], _f32r, kind="ExternalInput")
    brt_d = nc.dram_tensor("brt", [ND, P, K], _f32r, kind="ExternalInput")
    bit_d = nc.dram_tensor("bit", [ND, P, K], _f32r, kind="ExternalInput")
    art_d = nc.dram_tensor("art", [NK, P, DM], _f32r, kind="ExternalInput")
    ait_d = nc.dram_tensor("ait", [NK, P, DM], _f32r, kind="ExternalInput")
    cs_d = nc.dram_tensor("cs", [NK, P, SEQ], _f32, kind="ExternalInput")
    sn_d = nc.dram_tensor("sn", [NK, P, SEQ], _f32, kind="ExternalInput")
    yt_d = nc.dram_tensor("yt", [ND, P, SEQ], _f32, kind="ExternalOutput")
    with tile.TileContext(nc) as tc:
        _emit(tc, nc, (ut_d, brt_d, bit_d, art_d, ait_d, cs_d, sn_d, yt_d))
    nc.compile()
    return nc


def _host_prep(a_params, B_w, C_w):
    """Fold the DFT into B/C and build phase tables (float64 on host)."""
    n, half = NSTATE, K
    a = a_params.astype(np.float64)
    a_full = np.zeros(n)
    a_full[1:half] = a[:half - 1]
    a_full[half + 1:] = -a[:half - 1][::-1]
    omega = np.fft.fft(a_full).imag
    theta = -2.0 * np.arctan(omega)          # lambda_k = e^{i theta_k}

    Bf = np.fft.fft(B_w.astype(np.float64), axis=0)[:half + 1]      # (513, d)
    G = np.conj(np.fft.fft(C_w.astype(np.float64), axis=1))[:, :half + 1]

    Br = np.empty((K, DM))
    Bi = np.empty((K, DM))
    Br[0], Bi[0] = Bf[0].real, Bf[half].real   # DC + Nyquist packed, theta=0
    Br[1:], Bi[1:] = Bf[1:half].real, Bf[1:half].imag
    Ar = np.empty((DM, K))
    Ai = np.empty((DM, K))
    Ar[:, 0] = (1.0 / n) * G[:, 0].real
    Ai[:, 0] = (1.0 / n) * G[:, half].real
    Ar[:, 1:] = (2.0 / n) * G[:, 1:half].real
    Ai[:, 1:] = -(2.0 / n) * G[:, 1:half].imag
    th = theta[:half].copy()
    th[0] = 0.0

    ang = np.outer(th, np.arange(SEQ, dtype=np.float64))   # (K, SEQ)
    f32 = np.float32

    def tiles(m, p):          # (R, C) -> (R//p, p, C) contiguous f32
        return np.ascontiguousarray(m.reshape(m.shape[0] // p, p, m.shape[1]),
                                    dtype=f32)

    return {
        "brt": tiles(Br.T.copy(), P),       # (8, 128, 512)  BrT[d, k]
        "bit": tiles(Bi.T.copy(), P),
        "art": tiles(Ar.T.copy(), P),       # (4, 128, 1024) ArT[k, d]
        "ait": tiles(Ai.T.copy(), P),
        "cs": tiles(np.cos(ang), P),        # (4, 128, 2048)
        "sn": tiles(np.sin(ang), P),
    }


def _run(u, a_params, B_w, C_w, D, trace=False):
    global _COMPILED
    if _COMPILED is None:
        _COMPILED = _build()
    nc = _COMPILED
    shared = _host_prep(np.asarray(a_params), np.asarray(B_w), np.asarray(C_w))
    u = np.asarray(u)
    in_maps = []
    for b in range(BATCH):
        m = dict(shared)
        m["ut"] = np.ascontiguousarray(
            u[b].T.reshape(ND, P, SEQ), dtype=np.float32)
        in_maps.append(m)
    res = run_bass_kernel_spmd(nc, in_maps, core_ids=list(range(BATCH)),
                               trace=trace)
    y = np.empty((BATCH, SEQ, DM), dtype=np.float32)
    for b in range(BATCH):
        y[b] = res.results[b]["yt"].reshape(DM, SEQ).T
    y += np.asarray(D)[None, None, :] * u
    return y, res


def kernel(u, a_params, B_w, C_w, D):
    y, _ = _run(u, a_params, B_w, C_w, D)
    return y
